# revision 34
# baseline (speedup 1.0000x reference)
"""HeightmapNormalsLoss TRN2 kernel.

Data-parallel over 8 NeuronCores: 4 image-pairs per core. Per pair:
Sobel X,Y via TensorEngine band matmuls (vertical bands stationary, x4
scale folded into the weights; bf16 inputs host-cast), then a chain
split into 3 row-tile groups so ACT/DVE stages pipeline finely:

  s"  = X^2 + Y^2                  (custom DVE op w/ hand-written 2x_1P
                                    microcode: lo/hi bf16 pairs on
                                    parallel ALU chains, 2 elem/cyc)
  r   = Rsqrt(63/16 s" + 1)        (ACT Abs_reciprocal_sqrt; its table
                                    set also covers copy+abs -> one
                                    ACT_TABLE_LOAD total)
  nx  = X*r, ny = Y*r              (DVE TT, bf16 2x mode)
  nz  = (r - s"(s"/2048 + 1/32)r)  (custom DVE op; deg-2 poly of
                                    sqrt(1-s), max err 3e-5 since
                                    s <= 0.081)
  dd  = n_gen - n_tgt              (DVE TT)
  acc = sum |dd|                   (ACT Abs + accum)

Stationary blocks are zero-padded to 128 output rows so junk PSUM rows
are exactly 0 -> r=1, nz=1 on both images -> |dd|=0 (junk-safe, no
masking). Per-core output: [128, 12] f32 partial sums; host reduces.
"""
import sys

sys.path.insert(0, "/opt/trn_rl_repo")

import numpy as np
import ml_dtypes

H = W = 512
N_CORES = 8
PAIRS_PER_CORE = 4
TOTAL_B = 32
NT = 5

# (out_row_start, M, in_row_start, K, variant_idx)
ROW_TILES = [
    (0, 127, 0, 128, 0),
    (127, 126, 126, 128, 1),
    (253, 126, 252, 128, 1),
    (379, 126, 378, 128, 1),
    (505, 7, 504, 8, 2),
]


def _build_bands_np():
    """[128, 12*128] f32: blocks (band*3 + variant), bands sv, -sv, dv, 2dv.
    All entries x4 (folds the 4/len normalization into the matmul)."""
    mats = {}
    for v, (K, M) in enumerate([(128, 127), (128, 126), (8, 7)]):
        sv = np.zeros((128, 128), np.float32)
        dv = np.zeros((128, 128), np.float32)
        if v == 0:  # first: m=0 clamps row -1 -> 0
            sv[0, 0], sv[1, 0] = 3.0, 1.0
            dv[0, 0], dv[1, 0] = 1.0, -1.0
            for m in range(1, M):
                sv[m - 1, m], sv[m, m], sv[m + 1, m] = 1.0, 2.0, 1.0
                dv[m - 1, m], dv[m + 1, m] = 1.0, -1.0
        elif v == 1:  # mid
            for m in range(M):
                sv[m, m], sv[m + 1, m], sv[m + 2, m] = 1.0, 2.0, 1.0
                dv[m, m], dv[m + 2, m] = 1.0, -1.0
        else:  # last: m=M-1 (global 511) clamps row 512 -> 511
            for m in range(M - 1):
                sv[m, m], sv[m + 1, m], sv[m + 2, m] = 1.0, 2.0, 1.0
                dv[m, m], dv[m + 2, m] = 1.0, -1.0
            m = M - 1
            sv[m, m], sv[m + 1, m] = 1.0, 3.0
            dv[m, m], dv[m + 1, m] = 1.0, -1.0
        mats[(0, v)] = 4.0 * sv
        mats[(1, v)] = -4.0 * sv
        mats[(2, v)] = 4.0 * dv
        mats[(3, v)] = 8.0 * dv
    w = np.zeros((128, 14 * 128), np.float32)
    for b in range(4):
        for v in range(3):
            w[:, (b * 3 + v) * 128 : (b * 3 + v + 1) * 128] = mats[(b, v)]
    # stacked tail-tile blocks: the 3 column-shift windows live at partition
    # offsets 0/8/16 of the moving operand, so one matmul does the whole
    # horizontal conv. block 12: Y = [dv; dv; 2dv]; block 13: X = [sv; -sv]
    w[0:8, 12 * 128 : 13 * 128] = mats[(2, 2)][0:8]
    w[8:16, 12 * 128 : 13 * 128] = mats[(2, 2)][0:8]
    w[16:24, 12 * 128 : 13 * 128] = mats[(3, 2)][0:8]
    w[0:8, 13 * 128 : 14 * 128] = mats[(0, 2)][0:8]
    w[8:16, 13 * 128 : 14 * 128] = mats[(1, 2)][0:8]
    return w.astype(ml_dtypes.bfloat16)


_REG = {}


def _absdiff_uops_1x(spec, lower, ver):
    """3-uop 1x program: lower()'s [init, main] with main split into a
    1-cycle COUNT uop + the SRC_TENSOR_DONE remainder, so the 1x and 2x
    variants have equal uop counts (table-gen requirement)."""
    import copy
    from concourse.dve_uop import Trigger

    u_init, u_main = lower(spec, ver=ver)
    u_mid = copy.deepcopy(u_main)
    u_mid.trigger = (Trigger.COUNT, Trigger.NONE, Trigger.NONE)
    u_mid.repeat_count = 1
    u_mid.next_uop = (2, 0, 0)
    return [u_init, u_mid, copy.deepcopy(u_main)]


def _absdiff_uops_2x():
    """2x_1P |a-b| + add-accumulate. Body depth 3 (dlo, dhi, pair-sum),
    accumulator at blk3 (acc += ps via ADD(CURR, PREV)); two warmup uops
    feed zeros into the accumulator until the first pair-sum arrives.
    Outputs dlo/dhi ride delay lanes 0/1 to the packed write."""
    import copy
    from concourse.dve_uop import (
        UopConfig,
        UopDpConfig,
        InpSel,
        OutPath,
        OutSel,
        AluOp as UAluOp,
        AluInp,
        DelayInp,
        Trigger,
    )

    P = AluInp.PREV_ALU_OUT
    D0, D1, D2 = AluInp.PREV_DELAY_0, AluInp.PREV_DELAY_1, AluInp.PREV_DELAY_2
    D3, D4 = AluInp.PREV_DELAY_3, AluInp.PREV_DELAY_4
    KEEP, CAP = DelayInp.PREV_DELAY, DelayInp.PREV_ALU_OUT

    def blk(op, s0, s1, dsel, aA=0):
        d = [DelayInp.PREV_ALU_OUT] * 7
        de = [0] * 7
        for k, sel in dsel.items():
            d[k] = sel
            de[k] = 1
        b = UopDpConfig(op=op, alu_src0=s0, alu_src1=s1, delay=d)
        b.alu_out_enable = 1
        b.alu_out_a_enable = aA
        b.delay_enable = de
        return b

    AD, A, B = UAluOp.ABSOLUTE_DIFF, UAluOp.ADD, UAluOp.BYPASS

    def make(init, warm):
        dp = [
            blk(AD, P, D0, {1: KEEP, 2: KEEP, 3: KEEP, 4: KEEP}),      # dlo
            blk(AD, D1, D2, {0: CAP, 3: KEEP, 4: KEEP}),               # dhi
            (blk(B, D4, D4, {0: KEEP, 3: KEEP, 4: KEEP})               # ps=0
             if (init or warm) else
             blk(A, P, D0, {0: KEEP, 1: CAP, 3: KEEP, 4: KEEP})),      # ps
            (blk(B, D3, D3, {0: KEEP, 1: KEEP}, aA=1)                  # acc=C0
             if init else
             blk(A, AluInp.CURR_ALU_OUT, P, {0: KEEP, 1: KEEP}, aA=1)),
            blk(B, P, P, {0: KEEP, 1: KEEP}, aA=1),
            blk(B, P, P, {0: KEEP, 1: KEEP}, aA=1),
            blk(B, P, P, {0: KEEP, 1: KEEP}, aA=1),
            blk(B, P, P, {0: KEEP, 1: KEEP}, aA=1),
        ]
        u = UopConfig()
        u.inp = [
            InpSel.SRC_0,
            InpSel.SRC_1,
            InpSel.SRC_0_HI,
            InpSel.SRC_1_HI,
            InpSel.CONST_0,
            InpSel.ZERO,
            InpSel.ZERO,
            InpSel.ZERO,
        ]
        u.inp_enable = [1, 1, 1, 1, 1, 1, 0, 0]
        u.out = {
            OutPath.WR0_LO: OutSel.DELAY_0,
            OutPath.WR0_HI: OutSel.DELAY_1,
            OutPath.WR1_LO: OutSel.ALU_OUT,
            OutPath.WR1_HI: OutSel.ALU_OUT,
        }
        en = 0 if init else 1
        u.out_enable = {
            OutPath.WR0_LO: en,
            OutPath.WR0_HI: en,
            OutPath.WR1_LO: 0,
            OutPath.WR1_HI: 0,
        }
        u.accum_enabled = 1
        if init or warm:
            u.trigger = (Trigger.COUNT, Trigger.NONE, Trigger.NONE)
            u.repeat_count = 1
            u.next_uop = (1 if init else 2, 0, 0)
        else:
            u.trigger = (Trigger.SRC_TENSOR_DONE, Trigger.NONE, Trigger.NONE)
            u.next_uop = (0, 0, 0)
            u.require_inp0 = 1
            u.require_inp1 = 1
        u.datapath_config = dp
        return u

    return [make(True, False), make(False, True), make(False, False)]


def _absdiffb_uops_2x():
    """2x variant B: acc_lo at blk1 (template position), acc_hi at blk3,
    combined at blk4 via the delay-lane transport of acc_lo. Output tensor
    values are garbage (dead output); only accum_out matters."""
    from concourse.dve_uop import (
        UopConfig,
        UopDpConfig,
        InpSel,
        OutPath,
        OutSel,
        AluOp as UAluOp,
        AluInp,
        DelayInp,
        Trigger,
    )

    P = AluInp.PREV_ALU_OUT
    D0, D1, D2 = AluInp.PREV_DELAY_0, AluInp.PREV_DELAY_1, AluInp.PREV_DELAY_2
    D3, D4 = AluInp.PREV_DELAY_3, AluInp.PREV_DELAY_4
    CUR = AluInp.CURR_ALU_OUT
    KEEP, CAP = DelayInp.PREV_DELAY, DelayInp.PREV_ALU_OUT

    def blk(op, s0, s1, dsel, aA=0):
        d = [DelayInp.PREV_ALU_OUT] * 7
        de = [0] * 7
        for k, sel in dsel.items():
            d[k] = sel
            de[k] = 1
        b = UopDpConfig(op=op, alu_src0=s0, alu_src1=s1, delay=d)
        b.alu_out_enable = 1
        b.alu_out_a_enable = aA
        b.delay_enable = de
        return b

    AD, A, B = UAluOp.ABSOLUTE_DIFF, UAluOp.ADD, UAluOp.BYPASS
    XOR = UAluOp.BITWISE_XOR  # XOR(x, x) = 0 regardless of stale pipe state

    def make(init, warm):
        dp = [
            blk(AD, P, D0, {1: KEEP, 2: KEEP, 3: KEEP}),              # dlo
            (blk(B, D3, D3, {1: KEEP, 2: KEEP}, aA=1)                 # acc_lo=C0
             if init else
             blk(A, CUR, P, {1: KEEP, 2: KEEP}, aA=1)),
            (blk(XOR, P, P, {0: CAP}, aA=1)                           # dhi (0 in
             if (init or warm) else                                   #  warmup)
             blk(AD, D1, D2, {0: CAP}, aA=1)),
            (blk(XOR, P, P, {0: KEEP}, aA=1)                          # acc_hi=0
             if init else
             blk(A, CUR, P, {0: KEEP}, aA=1)),
            blk(A, D0, P, {0: KEEP}, aA=1),                           # combine
            blk(B, P, P, {0: KEEP}, aA=1),
            blk(B, P, P, {0: KEEP}, aA=1),
            blk(B, P, P, {0: KEEP}, aA=1),
        ]
        u = UopConfig()
        u.inp = [
            InpSel.SRC_0,
            InpSel.SRC_1,
            InpSel.SRC_0_HI,
            InpSel.SRC_1_HI,
            InpSel.CONST_0,
            InpSel.ZERO,
            InpSel.ZERO,
            InpSel.ZERO,
        ]
        u.inp_enable = [1, 1, 1, 1, 1, 1, 0, 0]
        u.out = {
            OutPath.WR0_LO: OutSel.ALU_OUT,
            OutPath.WR0_HI: OutSel.ALU_OUT,
            OutPath.WR1_LO: OutSel.ALU_OUT,
            OutPath.WR1_HI: OutSel.ALU_OUT,
        }
        en = 0 if init else 1
        u.out_enable = {
            OutPath.WR0_LO: en,
            OutPath.WR0_HI: en,
            OutPath.WR1_LO: 0,
            OutPath.WR1_HI: 0,
        }
        u.accum_enabled = 1
        if init or warm:
            u.trigger = (Trigger.COUNT, Trigger.NONE, Trigger.NONE)
            u.repeat_count = 1
            u.next_uop = (1 if init else 2, 0, 0)
        else:
            u.trigger = (Trigger.SRC_TENSOR_DONE, Trigger.NONE, Trigger.NONE)
            u.next_uop = (0, 0, 0)
            u.require_inp0 = 1
            u.require_inp1 = 1
        u.datapath_config = dp
        return u

    return [make(True, False), make(False, True), make(False, False)]


def _sqsum_uops_2x():
    """Hand-written 2x_1P program for SQSUM: lo/hi bf16 pairs processed by
    parallel ALU chains (mirrors the stock TENSOR_TENSOR 2x_1P slot).
    blocks: 0 xlo^2, 1 ylo^2, 2 LO=add, 3 xhi^2, 4 yhi^2, 5 HI=add,
    6-7 bypass carry HI in the ALU chain while LO rides delay lane 0."""
    from concourse.dve_uop import (
        UopConfig,
        UopDpConfig,
        InpSel,
        OutPath,
        OutSel,
        AluOp as UAluOp,
        AluInp,
        DelayInp,
        Trigger,
    )

    P, D = AluInp.PREV_ALU_OUT, DelayInp.PREV_ALU_OUT
    D0, D1, D2 = AluInp.PREV_DELAY_0, AluInp.PREV_DELAY_1, AluInp.PREV_DELAY_2
    KEEP = DelayInp.PREV_DELAY

    def blk(op, s0, s1, den, dsel, out_en=1):
        d = [DelayInp.PREV_ALU_OUT] * 7
        de = [0] * 7
        for k, sel in dsel.items():
            d[k] = sel
        for k in den:
            de[k] = 1
        b = UopDpConfig(op=op, alu_src0=s0, alu_src1=s1, delay=d)
        b.alu_out_enable = out_en
        b.delay_enable = de
        return b

    M, A, B = UAluOp.MULTIPLY, UAluOp.ADD, UAluOp.BYPASS
    dp = [
        blk(M, P, P, (0, 1, 2), {0: KEEP, 1: KEEP, 2: KEEP}),      # xlo^2
        blk(M, D0, D0, (0, 1, 2), {0: D, 1: KEEP, 2: KEEP}),       # ylo^2; d0<-xlo^2
        blk(A, D0, P, (1, 2), {1: KEEP, 2: KEEP}),                 # LO
        blk(M, D1, D1, (0, 2), {0: D, 2: KEEP}),                   # xhi^2; d0<-LO
        blk(M, D2, D2, (0, 1), {0: KEEP, 1: D}),                   # yhi^2; d1<-xhi^2
        blk(A, D1, P, (0,), {0: KEEP}),                            # HI
        blk(B, P, P, (0,), {0: KEEP}),
        blk(B, P, P, (0,), {0: KEEP}),
    ]
    u = UopConfig()
    u.inp = [
        InpSel.SRC_0,
        InpSel.SRC_1,
        InpSel.SRC_0_HI,
        InpSel.SRC_1_HI,
        InpSel.ZERO,
        InpSel.ZERO,
        InpSel.ZERO,
        InpSel.ZERO,
    ]
    u.inp_enable = [1, 1, 1, 1, 0, 0, 0, 0]
    u.out = {
        OutPath.WR0_LO: OutSel.DELAY_0,
        OutPath.WR0_HI: OutSel.ALU_OUT,
        OutPath.WR1_LO: OutSel.ALU_OUT,
        OutPath.WR1_HI: OutSel.ALU_OUT,
    }
    u.out_enable = {
        OutPath.WR0_LO: 1,
        OutPath.WR0_HI: 1,
        OutPath.WR1_LO: 0,
        OutPath.WR1_HI: 0,
    }
    u.require_inp0 = 1
    u.require_inp1 = 1
    u.trigger = (Trigger.SRC_TENSOR_DONE, Trigger.NONE, Trigger.NONE)
    u.next_uop = (0, 0, 0)
    u.datapath_config = dp
    return [u]


def _get_custom_ops():
    """Register the two fused DVE ops (once per process) and return them."""
    if _REG:
        return _REG
    from concourse import dve_ops as DO
    from concourse.dve_spec import Spec, Src0, Src1, C0, C1, C2, lower, sq, _has_src1
    from concourse.dve_uop import DveOpSpec
    from dataclasses import dataclass, field as _field

    b2x = {"ANT_SQSUM_HN": lambda sp, v: _sqsum_uops_2x(),
           "ANT_ABSDIFF2X_HN": lambda sp, v: _absdiff_uops_2x(),
           "ANT_ABSDIFFB_HN": lambda sp, v: _absdiffb_uops_2x()}
    b1x = {"ANT_ABSDIFF_HN": lambda sp, v: _absdiff_uops_1x(sp, lower, v),
           "ANT_ABSDIFF2X_HN": lambda sp, v: _absdiff_uops_1x(sp, lower, v),
           "ANT_ABSDIFFB_HN": lambda sp, v: _absdiff_uops_1x(sp, lower, v)}

    @dataclass(frozen=True)
    class DveOp2x(DO.DveOp):
        """DveOp whose v3 compile also carries a hand-written 2x_1P slot."""

        def compile(self, ver):
            key = (self.name, ver)
            cached = DO._COMPILE_CACHE.get(key)
            if cached is not None:
                return cached
            two_x = (b2x[self.name](self.spec, ver)
                     if (self.name in b2x and ver == "v3") else None)
            ones = (b1x[self.name](self.spec, ver) if self.name in b1x
                    else lower(self.spec, ver=ver))
            result = DveOpSpec(
                name=self.name,
                opcode=DO.get_dve_sub_opcode(self.name),
                uops=ones,
                rd1_en=True,
                uops_2x=two_x,
                perf_max=1 if two_x else 0,
            )
            got = result.sha(ver)
            if self.uops_sha.get(ver) != got:
                raise ValueError(f"{self.name}: sha drift {got}")
            DO._COMPILE_CACHE[key] = result
            return result

    def ref_sqsum(in0, in1, c0, c1, c2):
        return in0.astype(np.float32) ** 2 + in1.astype(np.float32) ** 2

    def ref_nzpr(in0, in1, c0, c1, c2):
        s = in0.astype(np.float32)
        r = in1.astype(np.float32)
        return r - s * (s * c0 + c1) * r

    from concourse.dve_spec import Bin, AluOp as SAluOp
    from operator import add as _opadd

    def ref_absdiff(in0, in1, c0, c1, c2):
        b = np.abs(in0.astype(np.float32) - in1.astype(np.float32))
        return b, c0 + b.reshape(b.shape[0], -1).sum(axis=-1, keepdims=True)

    defs = [
        ("ANT_SQSUM_HN", Spec(body=sq(Src0) + sq(Src1), reference=ref_sqsum)),
        (
            "ANT_NZPR_HN",
            Spec(body=Src1 - (Src0 * (Src0 * C0 + C1)) * Src1, reference=ref_nzpr),
        ),
        (
            "ANT_ABSDIFF_HN",
            Spec(
                body=Bin(SAluOp.ABSOLUTE_DIFF, Src0, Src1),
                accum=_opadd,
                accum_init=C0,
                reference=ref_absdiff,
            ),
        ),
        (
            "ANT_ABSDIFF2X_HN",
            Spec(
                body=Bin(SAluOp.ABSOLUTE_DIFF, Src0 * C1, Src1 * C1),
                accum=_opadd,
                accum_init=C0,
                reference=ref_absdiff,
            ),
        ),
        (
            "ANT_ABSDIFFB_HN",
            Spec(
                body=Bin(SAluOp.ABSOLUTE_DIFF, Src0 * C2, Src1 * C2),
                accum=_opadd,
                accum_init=C0,
                reference=ref_absdiff,
            ),
        ),
    ]
    for name, spec in defs:
        if name not in DO._SUB_OPCODE_FOR_NAME:
            row = DO._CUSTOM_DVE_ROW_BASE + len(DO.OPS)
            DO._SUB_OPCODE_FOR_NAME[name] = row
            is2x = name in b2x or name in b1x
            shas = {}
            for ver in ("v3", "v4"):
                uops = (b1x[name](spec, ver) if name in b1x
                        else lower(spec, ver=ver))
                two_x = (b2x[name](spec, ver)
                         if (name in b2x and ver == "v3") else None)
                shas[ver] = DveOpSpec(
                    name=name,
                    opcode=row,
                    uops=uops,
                    rd1_en=_has_src1(spec),
                    uops_2x=two_x,
                    perf_max=1 if two_x else 0,
                ).sha(ver)
            # keep sha consistent with DveOp2x.compile's construction
            cls = DveOp2x if is2x else DO.DveOp
            op = cls(name, spec, subdim=False, uops_sha=shas)
            DO.OPS.append(op)
            DO.CUSTOM_DVE_SPECS[name] = spec
        _REG[name] = next(o for o in DO.OPS if o.name == name)
    return _REG


def _kernel_body(tc, gen_d, tgt_d, w_d, acc_d):
    from contextlib import ExitStack
    from concourse import mybir

    ops = _get_custom_ops()
    nc = tc.nc
    AF = mybir.ActivationFunctionType
    OP = mybir.AluOpType
    f32 = mybir.dt.float32
    bf16 = mybir.dt.bfloat16

    with ExitStack() as ctx:
        persist = ctx.enter_context(tc.tile_pool(name="persist", bufs=1))
        xp_pool = ctx.enter_context(tc.tile_pool(name="xp", bufs=4))
        ps_pool = ctx.enter_context(tc.tile_pool(name="ps", bufs=2, space="PSUM"))
        c_pool = ctx.enter_context(tc.tile_pool(name="c", bufs=2))
        s_pool = ctx.enter_context(tc.tile_pool(name="s", bufs=2))
        r_pool = ctx.enter_context(tc.tile_pool(name="r", bufs=2))
        n_pool = ctx.enter_context(tc.tile_pool(name="n", bufs=2))
        d_pool = ctx.enter_context(tc.tile_pool(name="d", bufs=2))
        a_pool = ctx.enter_context(tc.tile_pool(name="a", bufs=1))

        wt = persist.tile([128, 14 * 128], bf16)
        nc.sync.dma_start(wt[:], w_d[:])
        accbuf = persist.tile([128, 3 * PAIRS_PER_CORE], f32)
        nc.vector.memset(accbuf[:], 0.0)
        bias0 = persist.tile([128, 1], f32)
        nc.vector.memset(bias0[:], 0.0)
        bias1 = persist.tile([128, 1], f32)
        nc.vector.memset(bias1[:], 1.0)

        def w_sl(band, v, K):
            blk = (band * 3 + v) * 128
            return wt[0:K, blk : blk + 128]

        for pair in range(PAIRS_PER_CORE):
            # pair-batched extracted Sobel responses: [row, ch, img, rt, col]
            # (img-major so custom-DVE operands merge to <=2 free dims)
            cp = c_pool.tile([128, 2, 2, NT, W], bf16, tag="cp", name="cp")
            for rt, (r0, M, i0, K, v) in enumerate(ROW_TILES):
                if rt == 4:
                    # tail tile: L/R/M shift windows stacked along the
                    # contraction dim at partition offsets 0/8/16
                    xp4 = xp_pool.tile([24, 2, W], bf16, tag="xp4", name="xp4")
                    for im, srcd in ((0, gen_d), (1, tgt_d)):
                        nc.sync.dma_start(
                            xp4[16:24, im, :], srcd[pair, i0 : i0 + 8, :]
                        )
                        nc.sync.dma_start(
                            xp4[0:8, im, 1:W], srcd[pair, i0 : i0 + 8, 0 : W - 1]
                        )
                        nc.sync.dma_start(
                            xp4[8:16, im, 0 : W - 1], srcd[pair, i0 : i0 + 8, 1:W]
                        )
                        # replication-pad edge columns via tiny DMAs (engine
                        # ops can't start at partition 8)
                        nc.sync.dma_start(
                            xp4[0:8, im, 0:1], srcd[pair, i0 : i0 + 8, 0:1]
                        )
                        nc.sync.dma_start(
                            xp4[8:16, im, W - 1 : W],
                            srcd[pair, i0 : i0 + 8, W - 1 : W],
                        )
                    pt = ps_pool.tile([128, 2, 2, W], f32, tag="pt")
                    for im in range(2):
                        nc.tensor.matmul(
                            pt[:, 1, im, :],
                            wt[0:24, 12 * 128 : 13 * 128],
                            xp4[0:24, im, :],
                            start=True,
                            stop=True,
                        )
                        nc.tensor.matmul(
                            pt[:, 0, im, :],
                            wt[0:16, 13 * 128 : 14 * 128],
                            xp4[0:16, im, :],
                            start=True,
                            stop=True,
                        )
                    nc.scalar.copy(cp[:, :, :, rt, :], pt[:, :, :, :])
                    continue
                xp = xp_pool.tile([128, 2, W + 2], bf16, tag="xp")
                nc.sync.dma_start(xp[0:K, 0, 1 : W + 1], gen_d[pair, i0 : i0 + K, :])
                nc.sync.dma_start(xp[0:K, 1, 1 : W + 1], tgt_d[pair, i0 : i0 + K, :])
                # replicate-pad edge columns (both images in one op)
                nc.vector.tensor_copy(xp[0:K, :, 0:1], xp[0:K, :, 1:2])
                nc.vector.tensor_copy(xp[0:K, :, W + 1 : W + 2], xp[0:K, :, W : W + 1])

                # pt: [128, ch, img, col]; full 128 out rows (junk rows = 0).
                # Per-image 2D matmuls (ISA caps mm free size at one PSUM
                # bank = 512 f32); same-stationary matmuls are consecutive.
                pt = ps_pool.tile([128, 2, 2, W], f32, tag="pt")
                plan = [
                    (2, 0, 1, True, False),   # Y += dv @ left
                    (2, 2, 1, False, False),  # Y += dv @ right
                    (3, 1, 1, False, True),   # Y += 2dv @ mid
                    (0, 0, 0, True, False),   # X += sv @ left
                    (1, 2, 0, False, True),   # X += -sv @ right
                ]
                for band, sh, co, st, sp in plan:
                    ws = w_sl(band, v, K)
                    for im in range(2):
                        nc.tensor.matmul(
                            pt[:, co, im, :],
                            ws,
                            xp[0:K, im, sh : sh + W],
                            start=st,
                            stop=sp,
                        )
                # extract psum f32 -> sbuf bf16, both images in one op
                nc.scalar.copy(cp[:, :, :, rt, :], pt[:, :, :, :])

            # pair-batched chain, split into three rt-groups so the
            # ACT<->DVE stage ping-pong pipelines at finer grain. Tiles are
            # img-major; .opt([0,1]) merges (rt, col) for the custom ops.
            for g, (lo, hi) in enumerate(((0, 2), (2, 4), (4, NT))):
                nt = hi - lo
                cx = cp[:, 0, :, lo:hi, :]
                cy = cp[:, 1, :, lo:hi, :]
                s2 = s_pool.tile([128, 2, nt, W], bf16, tag=f"s2{g}", name=f"s2{g}")
                nc.vector._custom_dve(
                    ops["ANT_SQSUM_HN"],
                    out=s2[:].opt([0, 1]),
                    in0=cx.opt([0, 1]),
                    in1=cy.opt([0, 1]),
                )
                rr = r_pool.tile([128, 2, nt, W], bf16, tag=f"rr{g}", name=f"rr{g}")
                nc.scalar.activation(
                    rr[:].opt([0, 1]),
                    s2[:].opt([0, 1]),
                    AF.Abs_reciprocal_sqrt,
                    bias=bias1[:, :],
                    scale=63.0 / 16.0,
                )
                nb = n_pool.tile(
                    [128, 2, 3, nt, W], bf16, tag=f"nb{g}", name=f"nb{g}"
                )
                nc.vector.tensor_tensor(nb[:, :, 0, :, :], cx, rr[:], OP.mult)
                nc.vector.tensor_tensor(nb[:, :, 1, :, :], cy, rr[:], OP.mult)
                nc.vector._custom_dve(
                    ops["ANT_NZPR_HN"],
                    out=nb[:, :, 2, :, :].opt([0, 1]),
                    in0=s2[:].opt([0, 1]),
                    in1=rr[:].opt([0, 1]),
                    s0=1.0 / 2048.0,
                    s1=1.0 / 32.0,
                )
                dt = d_pool.tile([128, 3, nt, W], bf16, tag=f"dt{g}", name=f"dt{g}")
                nc.vector.tensor_tensor(
                    dt[:].opt(),
                    nb[:, 0, :, :, :].opt(),
                    nb[:, 1, :, :, :].opt(),
                    OP.subtract,
                )
                at = a_pool.tile([128, 3, nt, W], bf16, tag=f"at{g}", name=f"at{g}")
                nc.scalar.activation(
                    at[:],
                    dt[:],
                    AF.Abs,
                    bias=bias0[:, :],
                    accum_out=accbuf[:, 3 * pair + g : 3 * pair + g + 1],
                )

        nc.sync.dma_start(acc_d[:], accbuf[:])


_CACHE = {}


def _get_module():
    if "nc" not in _CACHE:
        from concourse import bacc, tile, mybir

        nc = bacc.Bacc(
            "TRN2",
            target_bir_lowering=False,
            debug=False,
            enable_asserts=True,
            num_devices=N_CORES,
        )
        gen_d = nc.dram_tensor(
            "gen", (PAIRS_PER_CORE, H, W), mybir.dt.bfloat16, kind="ExternalInput"
        ).ap()
        tgt_d = nc.dram_tensor(
            "tgt", (PAIRS_PER_CORE, H, W), mybir.dt.bfloat16, kind="ExternalInput"
        ).ap()
        w_d = nc.dram_tensor(
            "w", (128, 14 * 128), mybir.dt.bfloat16, kind="ExternalInput"
        ).ap()
        acc_d = nc.dram_tensor(
            "acc", (128, 3 * PAIRS_PER_CORE), mybir.dt.float32, kind="ExternalOutput"
        ).ap()
        # perf_max must be set at instruction construction (rust field);
        # inject it for the 2x-capable SQSUM op while the kernel is built.
        from concourse import bass_isa as _bisa

        _orig_inst = _bisa.InstCustomDveAnt

        def _inst_with_perf(*a, **kw):
            if kw.get("op_name") in (
                "ANT_SQSUM_HN", "ANT_ABSDIFF2X_HN", "ANT_ABSDIFFB_HN"
            ):
                kw["perf_max"] = 1
            return _orig_inst(*a, **kw)

        _bisa.InstCustomDveAnt = _inst_with_perf
        try:
            with tile.TileContext(nc) as tc:
                _kernel_body(tc, gen_d, tgt_d, w_d, acc_d)
        finally:
            _bisa.InstCustomDveAnt = _orig_inst
        nc.compile()
        _CACHE["nc"] = nc
        _CACHE["w"] = _build_bands_np()
    return _CACHE["nc"], _CACHE["w"]


def _run(generated, target, **spmd_kwargs):
    from concourse import bass_utils

    nc, w = _get_module()
    g = np.asarray(generated, np.float32).reshape(TOTAL_B, H, W)
    t = np.asarray(target, np.float32).reshape(TOTAL_B, H, W)
    g = np.ascontiguousarray(g).astype(ml_dtypes.bfloat16)
    t = np.ascontiguousarray(t).astype(ml_dtypes.bfloat16)
    in_maps = [
        {
            "gen": g[c * PAIRS_PER_CORE : (c + 1) * PAIRS_PER_CORE],
            "tgt": t[c * PAIRS_PER_CORE : (c + 1) * PAIRS_PER_CORE],
            "w": w,
        }
        for c in range(N_CORES)
    ]
    return bass_utils.run_bass_kernel_spmd(
        nc, in_maps, core_ids=list(range(N_CORES)), **spmd_kwargs
    )


def kernel(generated, target):
    res = _run(generated, target)
    total = 0.0
    for r in res.results:
        total += float(np.asarray(r["acc"], np.float64).sum())
    return np.float32(total / (TOTAL_B * 3 * H * W))
